# revision 19
# baseline (speedup 1.0000x reference)
"""Causal self-attention Trainium2 kernel (8 NeuronCores, tensor-parallel heads).

Problem: B=4, T=2048, C=1024, H=16, D=64 fp32.
  q,k,v = x@Wq+bq, x@Wk+bk, x@Wv+bv  (per-head causal softmax(qk^T/8) @ v) @ Wp + bp

Sharding: 2 heads per core (column-split Wq/Wk/Wv, row-split Wp). Each core
computes a partial output [B*T, C]; host sums the 8 partials and adds bp.

v2 changes vs the fp32r baseline (414us):
  * bf16 everywhere except PSUM accumulation and the softmax denominator /
    normalize math: x, Wq/Wk/Wv/Wp, Q^T/K^T, V_aug, P (exp output), Y, and
    the output partial are all bf16.  Halves every DMA and SBUF stream,
    enables FWL on the 128-partition stationaries (proj + oproj), and gives
    2x DVE rate on the mask multiply.
  * Causal masking is a DVE 0/1-mask multiply on the exp output (the old
    accumulated bf16 Lm/Em mask matmuls are gone: fewer PE instructions and
    no stationary thrash on the diagonal).  Fully-masked padded columns of
    the o==3 tile multiply to exact zeros, so P row sums stay correct.
  * Normalize: reciprocal straight from the PSUM row (no ssum staging), both
    heads' broadcasts into one [128,RT] tile, and ONE full-width DVE multiply
    (f32 x f32 -> bf16) instead of two half-width ones.
  * Output projection pieces are queued a fixed ~3-tick delay after their
    block's normalize (not a whole block later), so the tail drains almost
    nothing; each piece's two 512-col halves are cast and written with a
    single [128,1024] bf16 DMA on the otherwise-idle sync queue.
  * xt row-tile loads are one 1MB DMA each (341GB/s regime, 1 trigger).

Per-core dataflow:
  xT [C, B*T] bf16 streamed from DRAM (host pre-transposes x).
  Q^T/K^T/V^T [128, B*T] = w.T @ xT  (w slices [C,128] bf16 stationary, FWL).
  V natural [tk,64] per head via one merged [128,128] PE transpose of V^T
  tiles (both heads at once); ones column appended -> V_aug [tk, 2, 65] so
  P@V_aug also yields softmax row sums.  v_aug is double-buffered by batch.
  S^T [tk, RT] = K^T.T @ Q^T per (batch, tq-block, tk-tile); both heads run
  concurrently as K=64 matmuls on distinct PE row groups into one 2-bank
  PSUM tile.  Diagonal tiles are narrowed to the visible query columns
  (widths 512/384/256/256); P^T = exp(S^T) via one ACT op (bf16 out), then
  the diagonal 128/256-column slab is multiplied by a precomputed 0/1
  triangle mask on the DVE.
  The attention inner loop is software-pipelined with lookahead (PV@tk-5
  after S@tk); batch b+1's projection work is sliced into atomic units and
  one unit is emitted per tk of batch b's attention, alongside one
  output-projection piece from ~3 ticks back.
  Y_aug^T [65, RT] accumulates over tk; row 64 is the softmax denominator.
  Both heads' accumulators are drained to SBUF right away, normalized with
  reciprocal_approx_fast + gpsimd partition_broadcast + one DVE multiply.
"""

import numpy as np

import concourse.tile as tile
from concourse import bacc, mybir
from concourse.bass_utils import run_bass_kernel_spmd

F32 = mybir.dt.float32
F32R = mybir.dt.float32r
BF16 = mybir.dt.bfloat16

B, T, C, H = 4, 2048, 1024, 16
D = C // H  # 64
N_CORES = 8
RT = 512  # row-tile (tq block) size
KT = C // 128  # 8 contraction tiles for projections


def build_kernel(n_batches=B):
    nc = bacc.Bacc(None, target_bir_lowering=False, debug=False)
    rows = n_batches * T
    bt_rt = T // RT  # 4 tq blocks per batch

    xT_d = nc.dram_tensor("xT", [C, rows], BF16, kind="ExternalInput")
    wq_d = nc.dram_tensor("wq", [C, 128], BF16, kind="ExternalInput")
    wk_d = nc.dram_tensor("wk", [C, 128], BF16, kind="ExternalInput")
    wv_d = nc.dram_tensor("wv", [C, 128], BF16, kind="ExternalInput")
    wp_d = nc.dram_tensor("wp", [128, C], BF16, kind="ExternalInput")
    bq_d = nc.dram_tensor("bq", [128], F32, kind="ExternalInput")
    bk_d = nc.dram_tensor("bk", [128], F32, kind="ExternalInput")
    bv_d = nc.dram_tensor("bv", [128], F32, kind="ExternalInput")
    mk_d = nc.dram_tensor("maskc", [128, 2, 256], BF16, kind="ExternalInput")
    id_d = nc.dram_tensor("ident", [128, 128], BF16, kind="ExternalInput")
    on_d = nc.dram_tensor("onescol", [128, 4 * (T // 128)], BF16, kind="ExternalInput")
    out_d = nc.dram_tensor("out", [rows, C], BF16, kind="ExternalOutput")

    n_vt = T // 128  # 16 v-tiles per batch

    with tile.TileContext(nc) as tc:
        with (
            nc.allow_low_precision(reason="bf16 intermediates are intentional"),
            tc.tile_pool(name="const", bufs=1) as const,
            tc.tile_pool(name="big", bufs=1) as big,
            tc.tile_pool(name="xs", bufs=3) as xs,
            tc.tile_pool(name="vt", bufs=2) as vtp,
            tc.tile_pool(name="pt", bufs=6) as ptp,
            tc.tile_pool(name="yt", bufs=4) as ytp,
            tc.tile_pool(name="nrm", bufs=2) as nrm,
            tc.tile_pool(name="ob", bufs=3) as ob,
            # PSUM: tag "s" slots are [128, 2, RT] = 2 banks (st tiles, proj
            # accumulators, V-transpose scratch and oproj pairs all rotate
            # through them). bufs=2 -> 4 banks. Tag "y" slots are 1 bank: the
            # two Y accumulators of the current block. Total 8 banks.
            tc.tile_pool(name="psS", bufs=2, space="PSUM") as psS,
            tc.tile_pool(name="psY", bufs=4, space="PSUM") as psY,
        ):
            # ---- constants ----
            wq = const.tile([128, KT, 128], BF16)
            wk = const.tile([128, KT, 128], BF16)
            wv = const.tile([128, KT, 128], BF16)
            wp = const.tile([128, C], BF16)
            wq_src = wq_d.rearrange("(k p) m -> p k m", p=128)
            nc.sync.dma_start(wq[:, 0:2, :], wq_src[:, 0:2, :])
            # ident first: the PE warmup loop is gated on it
            ident = const.tile([128, 128], BF16)
            maskc = const.tile([128, 2, 256], BF16)
            nc.gpsimd.dma_start(ident[:], id_d[:])
            nc.gpsimd.dma_start(maskc[:], mk_d[:])
            biases = []
            for name, d in (("bq", bq_d), ("bk", bk_d), ("bv", bv_d)):
                t = const.tile([128, 1], F32, name=f"{name}_sb")
                nc.gpsimd.dma_start(t[:], d.rearrange("(p o) -> p o", o=1))
                biases.append(t)
            nc.gpsimd.dma_start(wp[:], wp_d[:])

            # ---- whole-run big buffers ----
            n_rt_all = rows // RT
            qTs = [big.tile([128, RT], BF16, name=f"qT{i}") for i in range(n_rt_all)]
            kTs = [big.tile([128, RT], BF16, name=f"kT{i}") for i in range(n_rt_all)]
            # double-buffered by batch parity
            v_augs = [
                big.tile([128, n_vt, 2, 65], BF16, name=f"vaug{i}") for i in range(2)
            ]
            nc.gpsimd.memset(v_augs[0][:, :, :, 64:65], 1.0)
            nc.gpsimd.memset(v_augs[1][:, :, :, 64:65], 1.0)

            x_src = xT_d.rearrange("(k p) r -> p k r", p=128)
            nc.sync.dma_start(wk[:], wk_d.rearrange("(k p) m -> p k m", p=128))
            nc.sync.dma_start(wq[:, 2:8, :], wq_src[:, 2:8, :])
            nc.sync.dma_start(wv[:], wv_d.rearrange("(k p) m -> p k m", p=128))

            # PE warmup: bridges the initial DMA window and warms the HAM
            # clock gate before the first projection matmuls
            warm = psS.tile([128, RT], F32, name="warm", tag="s")
            for _ in range(48):
                nc.tensor.matmul(warm[:, 0:128], ident[:], ident[:], start=True, stop=True)

            # ---- projection work, sliced into atomic units ----
            def proj_units(b):
                """List of closures: xt DMA loads, 8-matmul w-groups (+bias),
                merged V-transposes, for batch b. Called one-per-tk during
                batch b-1's attention (or inline for batch 0)."""
                r0 = b * T
                units = []
                state = [None] * bt_rt  # xt tile per rowtile
                vstate = [None] * bt_rt  # vt_sb per rowtile

                def mk_xt(rt):
                    def u():
                        xt = xs.tile([128, KT, RT], BF16, name="xt")
                        c0r = r0 + rt * RT
                        nc.sync.dma_start(xt[:], x_src[:, :, c0r : c0r + RT])
                        state[rt] = xt

                    return u

                def mk_grp(rt, wi):
                    def u():
                        w, bias = ((wq, biases[0]), (wk, biases[1]), (wv, biases[2]))[
                            wi
                        ]
                        xt = state[rt]
                        acc = psS.tile([128, RT], F32, name="proj", tag="s")
                        for k in range(KT):
                            nc.tensor.matmul(
                                acc[:],
                                w[:, k, :],
                                xt[:, k, :],
                                start=(k == 0),
                                stop=(k == KT - 1),
                            )
                        grt = (r0 + rt * RT) // RT
                        if wi == 0:
                            nc.vector.tensor_scalar_add(qTs[grt][:], acc[:], bias[:])
                        elif wi == 1:
                            nc.vector.tensor_scalar_add(kTs[grt][:], acc[:], bias[:])
                        else:
                            vt_sb = vtp.tile([128, RT], BF16, name="vt_sb")
                            nc.vector.tensor_scalar_add(vt_sb[:], acc[:], bias[:])
                            vstate[rt] = vt_sb

                    return u

                def mk_vtrans(rt, c):
                    def u():
                        vt_sb = vstate[rt]
                        vtile = rt * (RT // 128) + c
                        vps = psS.tile([128, 2, 64], BF16, name="vps", tag="s")
                        nc.tensor.transpose(
                            vps[:], vt_sb[:, c * 128 : c * 128 + 128], ident[:]
                        )
                        nc.vector.tensor_copy(
                            v_augs[b % 2][:, vtile, :, 0:64], vps[:]
                        )

                    return u

                # xt DMA units lead their consumer groups by ~10 units so the
                # 1MB transfer completes before the matmuls need it
                units.append(mk_xt(0))
                units.append(mk_xt(1))
                for rt in range(bt_rt):
                    for wi in range(3):
                        units.append(mk_grp(rt, wi))
                    if rt < 2:
                        units.append(mk_xt(rt + 2))
                    for c in range(RT // 128):
                        units.append(mk_vtrans(rt, c))
                return units

            # oproj pieces: (ready_tick, yt, q0, rr); drained max 1 per tick
            oproj_q = []
            tick = [0]
            npieces = [0]

            def emit_piece(yt, q0, rr):
                ops2 = psS.tile([128, 2, 512], F32, name="ops2", tag="s")
                for nn in range(C // 512):
                    nc.tensor.matmul(
                        ops2[:, nn, :],
                        yt[:, rr * 128 : rr * 128 + 128],
                        wp[:, nn * 512 : nn * 512 + 512],
                        start=True,
                        stop=True,
                    )
                osb = ob.tile([128, 2, 512], BF16, name="osb")
                nc.vector.tensor_copy(osb[:], ops2[:])
                npieces[0] += 1
                dst = out_d.rearrange("r (n c) -> r n c", n=2)
                nc.gpsimd.dma_start(
                    dst[q0 + rr * 128 : q0 + rr * 128 + 128, :, :], osb[:]
                )

            def drain_piece():
                if oproj_q and oproj_q[0][0] <= tick[0]:
                    _, yt, q0, rr = oproj_q.pop(0)
                    emit_piece(yt, q0, rr)

            # batch 0's projections run standalone up front
            for u in proj_units(0):
                u()

            # Cross-block attention pipeline. pend entries carry their own
            # block context so PV matmuls (and each block's normalize, hooked
            # onto its last PV) drain during the NEXT block's S stream.
            pend = []  # (emit_pv, tk, pt, c0, fin)

            def pop_pend():
                emit_pv, tk, pt, c0, fin = pend.pop(0)
                emit_pv(tk, pt, c0)
                if fin is not None:
                    fin()

            def mk_normalize(yps, q0):
                def fin():
                    # Drain both accumulators out of PSUM (copies + reciprocal
                    # straight from the denominator row), then one broadcast
                    # tile and ONE full-width multiply.
                    yt = ytp.tile([128, RT], BF16, name="yt")
                    srows = [nrm.tile([1, RT], F32, name=f"srow{h}") for h in range(2)]
                    ssums = [nrm.tile([1, RT], F32, name=f"ssum{h}") for h in range(2)]
                    bcs = [nrm.tile([64, RT], F32, name=f"bc{h}") for h in range(2)]
                    yacs = [nrm.tile([64, RT], F32, name=f"yac{h}") for h in range(2)]
                    for h in range(2):
                        nc.vector.tensor_copy(ssums[h][:], yps[h][64:65, :])
                        nc.vector.tensor_copy(yacs[h][:], yps[h][0:64, :])
                        nc.vector.reciprocal_approx_fast(srows[h][:], ssums[h][:])
                    for h in range(2):
                        nc.gpsimd.partition_broadcast(bcs[h][:], srows[h][:])
                        nc.vector.tensor_mul(
                            yt[64 * h : 64 * h + 64, :], yacs[h][:], bcs[h][:]
                        )
                    for rr in range(RT // 128):
                        oproj_q.append((tick[0] + 3, yt, q0, rr))

                return fin

            for b in range(n_batches):
                r0 = b * T
                punits = proj_units(b + 1) if b + 1 < n_batches else []
                for tqb in range(bt_rt):
                    with nc.named_scope(f"attn{b}_{tqb}"):
                        if tqb == 0:
                            # kick off next batch's first xt prefetches early
                            for _ in range(2):
                                if punits:
                                    punits.pop(0)()
                        q0 = r0 + tqb * RT
                        n_tk = (tqb + 1) * (RT // 128)
                        qt_tile = qTs[q0 // RT]
                        yps = [
                            psY.tile([65, RT], F32, name=f"yacc{h}", tag="y")
                            for h in range(2)
                        ]

                        def emit_pv(tk, pt, c0, yps=yps, n_tk=n_tk, b=b):
                            for h in range(2):
                                nc.tensor.matmul(
                                    yps[h][:, c0:RT],
                                    v_augs[b % 2][:, tk, h, :],
                                    pt[:, h, c0:RT],
                                    start=(tk == 0),
                                    stop=(tk == n_tk - 1),
                                    skip_group_check=True,
                                )

                        fin = mk_normalize(yps, q0)
                        for tk in range(n_tk):
                            k0 = r0 + tk * 128
                            o = tk - tqb * (RT // 128)  # diag offset, >=0 on diag
                            diag = o >= 0
                            # narrowed query range for diagonal tiles (bf16
                            # matmuls run full rate at any N, so no padding)
                            c0 = 0 if not diag else o * 128
                            st = psS.tile([128, 2, RT], F32, name="st", tag="s")
                            kt_tile = kTs[k0 // RT]
                            kk = k0 % RT
                            for h in range(2):
                                hs = slice(64 * h, 64 * h + 64)
                                nc.tensor.matmul(
                                    st[:, h, c0:RT],
                                    kt_tile[hs, kk : kk + 128],
                                    qt_tile[hs, c0:RT],
                                    start=True,
                                    stop=True,
                                    skip_group_check=True,
                                )
                            pt = ptp.tile([128, 2, RT], BF16, name="pt")
                            nc.scalar.activation(
                                pt[:, :, c0:RT],
                                st[:, :, c0:RT],
                                mybir.ActivationFunctionType.Exp,
                            )
                            if diag:
                                nc.vector.tensor_mul(
                                    pt[:, :, c0 : c0 + 128],
                                    pt[:, :, c0 : c0 + 128],
                                    maskc[:, :, 128:256],
                                )
                            pend.append(
                                (emit_pv, tk, pt, c0, fin if tk == n_tk - 1 else None)
                            )
                            if punits:
                                punits.pop(0)()
                            drain_piece()
                            lim = 2 if b == n_batches - 1 else 4
                            if len(pend) > lim:
                                pop_pend()
                            if b == n_batches - 1:
                                drain_piece()
                                if len(pend) > lim:
                                    pop_pend()
                            tick[0] += 1
                # flush any projection units the attention loop didn't host
                for u in punits:
                    u()
            while pend:
                pop_pend()
                drain_piece()
                tick[0] += 1
            while oproj_q:
                _, yt, q0, rr = oproj_q.pop(0)
                emit_piece(yt, q0, rr)
    nc.compile()
    return nc


def make_mask():
    """0/1 mask, [128, 2, 256] (duplicated over the head dim).  Columns
    128..255 hold the within-diagonal-block triangle (key k visible to query
    column c iff k <= c); columns 0..127 are zeros (the fully-masked padded
    region of the o==3 diagonal tile)."""
    m = np.zeros((128, 2, 256), np.float32)
    for k in range(128):
        m[k, :, 128 + k :] = 1.0
    return m


def make_inputs_for_core(c, shared, Wq, bq, Wk, bk, Wv, bv, Wp):
    import ml_dtypes

    bf = ml_dtypes.bfloat16
    cols = slice(c * 128, (c + 1) * 128)
    return {
        "xT": shared["xT"],
        "wq": np.ascontiguousarray(np.asarray(Wq, np.float32)[:, cols] / 8.0).astype(bf),
        "wk": np.ascontiguousarray(np.asarray(Wk, np.float32)[:, cols]).astype(bf),
        "wv": np.ascontiguousarray(np.asarray(Wv, np.float32)[:, cols]).astype(bf),
        "wp": np.ascontiguousarray(np.asarray(Wp, np.float32)[cols, :]).astype(bf),
        "bq": np.ascontiguousarray(np.asarray(bq, np.float32)[cols] / 8.0),
        "bk": np.ascontiguousarray(np.asarray(bk, np.float32)[cols]),
        "bv": np.ascontiguousarray(np.asarray(bv, np.float32)[cols]),
        "maskc": shared["maskc"],
        "ident": shared["ident"],
        "onescol": shared["onescol"],
    }


def kernel(x, Wq, bq, Wk, bk, Wv, bv, Wp, bp, _nc_cache={}, **run_kwargs):
    import ml_dtypes

    bf = ml_dtypes.bfloat16
    n_batches = B
    if "nc" not in _nc_cache:
        _nc_cache["nc"] = build_kernel(n_batches)
    nc = _nc_cache["nc"]
    shared = {
        "xT": np.ascontiguousarray(
            np.asarray(x, np.float32).reshape(B * T, C).T
        ).astype(bf),
        "maskc": make_mask().astype(bf),
        "ident": np.eye(128, dtype=np.float32).astype(bf),
        "onescol": np.ones((128, 4 * (T // 128)), np.float32).astype(bf),
    }
    in_maps = [
        make_inputs_for_core(c, shared, Wq, bq, Wk, bk, Wv, bv, Wp)
        for c in range(N_CORES)
    ]
    res = run_bass_kernel_spmd(nc, in_maps, core_ids=list(range(N_CORES)), **run_kwargs)
    out = np.zeros((B * T, C), np.float32)
    for r in res.results:
        out += np.asarray(r["out"], np.float32)
    out += np.asarray(bp, np.float32)[None, :]
    if run_kwargs.get("trace"):
        kernel.last_result = res
    return out.reshape(B, T, C)
